# revision 16
# baseline (speedup 1.0000x reference)
"""Collider binary-tree sampling kernel for Trainium2 (8 NeuronCores).

out[:, 0:256]           = eps[:, 0:256]
level d (width w=2^(8-d), d=1..7):
out[:, off:off+w]       = prev[:, 0::2] + prev[:, 1::2] + 0.1*eps[:, off:off+w]

Pure data parallel: rows sharded 8 ways; per core the tree recursion runs
in SBUF, rows on partitions, levels along the free dim (in-place).
"""

import os
from contextlib import ExitStack

import numpy as np

import concourse.bacc as bacc
import concourse.bass as bass
import concourse.mybir as mybir
import concourse.tile as tile
from concourse.bass_utils import run_bass_kernel_spmd

DEPTH = 9
SIGMA = 0.1
TOTAL = 510  # 2**DEPTH - 2
LVL0 = 256
N_CORES = 8
N_ROWS = 262144
ROWS_PER_CORE = N_ROWS // N_CORES  # 32768
P = 128  # SBUF partitions


def build_nc(
    rows: int = ROWS_PER_CORE, t_batch: int = 8, bufs: int = 4, passes: int = 1
) -> bass.Bass:
    """Build one core's program for a [rows, 510] shard.

    passes>1 repeats the identical computation inside one NEFF (used only to
    measure per-pass HW time by slope; the real kernel uses passes=1).
    """
    assert rows % (P * t_batch) == 0
    nblk = rows // (P * t_batch)
    fp32 = mybir.dt.float32
    nc = bacc.Bacc(None, target_bir_lowering=False)
    eps = nc.dram_tensor("eps", [rows, TOTAL], fp32, kind="ExternalInput")
    out = nc.dram_tensor("out", [rows, TOTAL], fp32, kind="ExternalOutput")
    # row r = (b*P + p)*T + t  ->  partition p holds T *consecutive* rows, so
    # each partition's DMA segment is one contiguous T*510*4-byte DRAM chunk
    eps_r = eps.rearrange("(b p t) v -> b p t v", t=t_batch, p=P)
    out_r = out.rearrange("(b p t) v -> b p t v", t=t_batch, p=P)

    with ExitStack() as ctx:
        tc = ctx.enter_context(tile.TileContext(nc))
        pool = ctx.enter_context(tc.tile_pool(name="x", bufs=bufs))
        tpool = ctx.enter_context(tc.tile_pool(name="tmp", bufs=2))
        for _rep in range(passes):
            for b in range(nblk):
                x = pool.tile([P, t_batch, TOTAL], fp32, tag="x")
                nc.sync.dma_start(out=x[:], in_=eps_r[b])
                # scale all noise columns by sigma once (DVE, in-place: keeps
                # every intra-block dep on one engine so no instruction needs
                # more than one sem wait)
                nc.vector.tensor_scalar_mul(x[:, :, LVL0:TOTAL], x[:, :, LVL0:TOTAL], SIGMA)
                off, w = 0, LVL0
                for _ in range(1, DEPTH - 1):
                    tmp = tpool.tile([P, t_batch, w // 2], fp32, tag="tmp")
                    nc.vector.tensor_add(
                        tmp[:],
                        x[:, :, off : off + w : 2],
                        x[:, :, off + 1 : off + w : 2],
                    )
                    off, w = off + w, w // 2
                    nc.vector.tensor_add(
                        x[:, :, off : off + w], tmp[:], x[:, :, off : off + w]
                    )
                # store on the scalar engine's HWDGE ring: loads and stores on
                # separate rings stream concurrently instead of alternating
                nc.scalar.dma_start(out=out_r[b], in_=x[:])
    strip_implied_waits(nc)
    nc.compile()  # Bacc: splits any remaining multi-waits into event semaphores
    return nc


def strip_implied_waits(nc: bass.Bass) -> None:
    """Remove semaphore waits that are transitively implied.

    The neuronxcc in this container encodes at most ONE sync-wait per HW
    instruction (TT / TS / DMA structs reject more).  Tile's add_semaphores is
    per-engine minimal but not transitively minimal, so a consumer re-waits on
    a producer its own (kept) waits already imply.  Model: an instruction
    issues only after its attached waits pass and after the previous
    instruction on its engine stream issued; a semaphore reaching threshold v
    implies the minimal prefix of its (program-ordered, single-engine)
    updaters completed.  Any wait whose condition is already guaranteed at
    issue time is deleted.  Deleting implied waits never changes sync
    semantics.
    """
    f = nc.m.functions[0]
    insts = [i for b in f.blocks for i in b.instructions]
    n = len(insts)

    # per-sem cumulative update schedule in program order.  A sem is usable
    # only if every update is a positive add/inc AND all updaters are on one
    # engine (in-order completion); barrier sems (sub/dec, multi-engine) are
    # poisoned: their waits are never removed and contribute no knowledge.
    upd: dict = {}
    sem_engines: dict = {}
    poisoned: set = set()
    for k, i in enumerate(insts):
        si = i.sync_info
        if si is None:
            continue
        for u in si.on_update or []:
            good = u.sync_type == "semaphore" and u.update_mode in ("sem-add-imm", "sem-inc")
            if not good or not u.update_value or u.update_value <= 0:
                poisoned.add(u.ant_name)
            sem_engines.setdefault(u.ant_name, set()).add(str(i.engine))
            upd.setdefault(u.ant_name, []).append((k, u.update_value if good else None))
    for s, engs in sem_engines.items():
        if len(engs) > 1:
            poisoned.add(s)
    cum: dict = {}
    for s, lst in upd.items():
        if s in poisoned:
            cum[s] = [(k, None) for k, _ in lst]
            continue
        c, out = 0, []
        for k, v in lst:
            c = None if (v is None or c is None) else c + v
            out.append((k, c))
        cum[s] = out

    def producers(sem, v):
        pre = []
        for k, c in cum.get(sem, []):
            pre.append(k)
            if c is None:
                return None
            if c >= v:
                return pre
        return None

    # dependency DAG: per-engine stream edges (issue order) + wait->producer
    # edges; process in topological order so forward-listed producers credit.
    all_waits: list = []
    wprods: list = []
    preds: list = [set() for _ in range(n)]
    last_on_engine: dict = {}
    for k, i in enumerate(insts):
        eng = i.engine
        if eng in last_on_engine:
            preds[k].add(last_on_engine[eng])
        last_on_engine[eng] = k
        si = i.sync_info
        waits = list(si.on_wait) if (si and si.on_wait) else []
        all_waits.append(waits)
        wp = []
        for w in waits:
            ok = (
                w.sync_type == "semaphore"
                and w.wait_mode == "sem-ge-imm"
                and w.wait_value is not None
                and w.wait_value > 0
                and w.ant_name not in poisoned
            )
            ps = producers(w.ant_name, w.wait_value) if ok else None
            wp.append(ps)
            if ps is not None:
                preds[k].update(ps)
        wprods.append(wp)

    indeg = [0] * n
    succ: list = [[] for _ in range(n)]
    for k in range(n):
        for p in preds[k]:
            succ[p].append(k)
            indeg[k] += 1
    from collections import deque

    topo, q = [], deque(k for k in range(n) if indeg[k] == 0)
    while q:
        k = q.popleft()
        topo.append(k)
        for s2 in succ[k]:
            indeg[s2] -= 1
            if indeg[s2] == 0:
                q.append(s2)
    if len(topo) != n:
        return  # unexpected cycle: leave program untouched

    F = [0] * n  # bitset: instructions known COMPLETED when inst k issues
    C = [0] * n  # completed-set implied by inst k's completion (incl. itself)
    stream_pred: dict = {}
    seen_eng: dict = {}
    for k, i in enumerate(insts):
        eng = i.engine
        stream_pred[k] = seen_eng.get(eng)
        seen_eng[eng] = k

    for k in topo:
        i = insts[k]
        waits = all_waits[k]
        sp = stream_pred[k]
        base = F[sp] if sp is not None else 0
        wsets = []
        for ps in wprods[k]:
            if ps is None:
                wsets.append(None)
            else:
                s = 0
                for p in ps:
                    s |= C[p]
                wsets.append(s)
        keep = list(range(len(waits)))
        changed = True
        while changed:
            changed = False
            for j in list(keep):
                w = waits[j]
                if wprods[k][j] is None:
                    continue
                known = base
                for j2 in keep:
                    if j2 != j and wsets[j2] is not None:
                        known |= wsets[j2]
                credit = 0
                for ki, v in upd.get(w.ant_name, []):
                    if v is not None and (known >> ki) & 1:
                        credit += v
                if credit >= w.wait_value:
                    keep.remove(j)
                    changed = True
                    break
        fac = base
        for j in keep:
            if wsets[j] is not None:
                fac |= wsets[j]
        F[k] = fac
        C[k] = fac | (1 << k)
        if len(keep) != len(waits):
            si = i.sync_info
            i.sync_info = mybir.SyncInfo(
                on_wait=[waits[j] for j in keep], on_update=list(si.on_update or [])
            )


_CACHE: dict = {}
last_result = None  # BassKernelResults of the most recent run (for profiling)


def kernel(eps: np.ndarray, N=None, **_unused) -> np.ndarray:
    global last_result
    eps = np.ascontiguousarray(np.asarray(eps, dtype=np.float32))
    n = eps.shape[0]
    assert n % N_CORES == 0 and eps.shape[1] == TOTAL
    shard = n // N_CORES

    nc = _CACHE.get(("nc", shard))
    if nc is None:
        nc = _CACHE[("nc", shard)] = build_nc(rows=shard)

    in_maps = [{"eps": eps[i * shard : (i + 1) * shard]} for i in range(N_CORES)]
    res = run_bass_kernel_spmd(nc, in_maps, list(range(N_CORES)))
    last_result = res
    return np.concatenate([r["out"] for r in res.results], axis=0)


# revision 17
# speedup vs baseline: 1.0658x; 1.0658x over previous
"""Collider binary-tree sampling kernel for Trainium2 (8 NeuronCores).

out[:, 0:256]           = eps[:, 0:256]
level d (width w=2^(8-d), d=1..7):
out[:, off:off+w]       = prev[:, 0::2] + prev[:, 1::2] + 0.1*eps[:, off:off+w]

Pure data parallel: rows sharded 8 ways; per core the tree recursion runs
in SBUF, rows on partitions, levels along the free dim (in-place).
"""

import os
from contextlib import ExitStack

import numpy as np

import concourse.bacc as bacc
import concourse.bass as bass
import concourse.mybir as mybir
import concourse.tile as tile
from concourse.bass_utils import run_bass_kernel_spmd

DEPTH = 9
SIGMA = 0.1
TOTAL = 510  # 2**DEPTH - 2
LVL0 = 256
N_CORES = 8
N_ROWS = 262144
ROWS_PER_CORE = N_ROWS // N_CORES  # 32768
P = 128  # SBUF partitions


def build_nc(
    rows: int = ROWS_PER_CORE, t_batch: int = 16, bufs: int = 3, passes: int = 1
) -> bass.Bass:
    """Build one core's program for a [rows, 510] shard.

    passes>1 repeats the identical computation inside one NEFF (used only to
    measure per-pass HW time by slope; the real kernel uses passes=1).
    """
    assert rows % (P * t_batch) == 0
    nblk = rows // (P * t_batch)
    fp32 = mybir.dt.float32
    nc = bacc.Bacc(None, target_bir_lowering=False)
    eps = nc.dram_tensor("eps", [rows, TOTAL], fp32, kind="ExternalInput")
    out = nc.dram_tensor("out", [rows, TOTAL], fp32, kind="ExternalOutput")
    # row r = (b*P + p)*T + t  ->  partition p holds T *consecutive* rows, so
    # each partition's DMA segment is one contiguous T*510*4-byte DRAM chunk
    eps_r = eps.rearrange("(b p t) v -> b p t v", t=t_batch, p=P)
    out_r = out.rearrange("(b p t) v -> b p t v", t=t_batch, p=P)

    with ExitStack() as ctx:
        tc = ctx.enter_context(tile.TileContext(nc))
        pool = ctx.enter_context(tc.tile_pool(name="x", bufs=bufs))
        tpool = ctx.enter_context(tc.tile_pool(name="tmp", bufs=2))
        for _rep in range(passes):
            for b in range(nblk):
                x = pool.tile([P, t_batch, TOTAL], fp32, tag="x")
                nc.sync.dma_start(out=x[:], in_=eps_r[b])
                # scale all noise columns by sigma once (DVE, in-place: keeps
                # every intra-block dep on one engine so no instruction needs
                # more than one sem wait)
                nc.vector.tensor_scalar_mul(x[:, :, LVL0:TOTAL], x[:, :, LVL0:TOTAL], SIGMA)
                off, w = 0, LVL0
                for _ in range(1, DEPTH - 1):
                    tmp = tpool.tile([P, t_batch, w // 2], fp32, tag="tmp")
                    nc.vector.tensor_add(
                        tmp[:],
                        x[:, :, off : off + w : 2],
                        x[:, :, off + 1 : off + w : 2],
                    )
                    off, w = off + w, w // 2
                    nc.vector.tensor_add(
                        x[:, :, off : off + w], tmp[:], x[:, :, off : off + w]
                    )
                # store on the scalar engine's HWDGE ring: loads and stores on
                # separate rings stream concurrently instead of alternating
                nc.scalar.dma_start(out=out_r[b], in_=x[:])
    strip_implied_waits(nc)
    nc.compile()  # Bacc: splits any remaining multi-waits into event semaphores
    return nc


def strip_implied_waits(nc: bass.Bass) -> None:
    """Remove semaphore waits that are transitively implied.

    The neuronxcc in this container encodes at most ONE sync-wait per HW
    instruction (TT / TS / DMA structs reject more).  Tile's add_semaphores is
    per-engine minimal but not transitively minimal, so a consumer re-waits on
    a producer its own (kept) waits already imply.  Model: an instruction
    issues only after its attached waits pass and after the previous
    instruction on its engine stream issued; a semaphore reaching threshold v
    implies the minimal prefix of its (program-ordered, single-engine)
    updaters completed.  Any wait whose condition is already guaranteed at
    issue time is deleted.  Deleting implied waits never changes sync
    semantics.
    """
    f = nc.m.functions[0]
    insts = [i for b in f.blocks for i in b.instructions]
    n = len(insts)

    # per-sem cumulative update schedule in program order.  A sem is usable
    # only if every update is a positive add/inc AND all updaters are on one
    # engine (in-order completion); barrier sems (sub/dec, multi-engine) are
    # poisoned: their waits are never removed and contribute no knowledge.
    upd: dict = {}
    sem_engines: dict = {}
    poisoned: set = set()
    for k, i in enumerate(insts):
        si = i.sync_info
        if si is None:
            continue
        for u in si.on_update or []:
            good = u.sync_type == "semaphore" and u.update_mode in ("sem-add-imm", "sem-inc")
            if not good or not u.update_value or u.update_value <= 0:
                poisoned.add(u.ant_name)
            sem_engines.setdefault(u.ant_name, set()).add(str(i.engine))
            upd.setdefault(u.ant_name, []).append((k, u.update_value if good else None))
    for s, engs in sem_engines.items():
        if len(engs) > 1:
            poisoned.add(s)
    cum: dict = {}
    for s, lst in upd.items():
        if s in poisoned:
            cum[s] = [(k, None) for k, _ in lst]
            continue
        c, out = 0, []
        for k, v in lst:
            c = None if (v is None or c is None) else c + v
            out.append((k, c))
        cum[s] = out

    def producers(sem, v):
        pre = []
        for k, c in cum.get(sem, []):
            pre.append(k)
            if c is None:
                return None
            if c >= v:
                return pre
        return None

    # dependency DAG: per-engine stream edges (issue order) + wait->producer
    # edges; process in topological order so forward-listed producers credit.
    all_waits: list = []
    wprods: list = []
    preds: list = [set() for _ in range(n)]
    last_on_engine: dict = {}
    for k, i in enumerate(insts):
        eng = i.engine
        if eng in last_on_engine:
            preds[k].add(last_on_engine[eng])
        last_on_engine[eng] = k
        si = i.sync_info
        waits = list(si.on_wait) if (si and si.on_wait) else []
        all_waits.append(waits)
        wp = []
        for w in waits:
            ok = (
                w.sync_type == "semaphore"
                and w.wait_mode == "sem-ge-imm"
                and w.wait_value is not None
                and w.wait_value > 0
                and w.ant_name not in poisoned
            )
            ps = producers(w.ant_name, w.wait_value) if ok else None
            wp.append(ps)
            if ps is not None:
                preds[k].update(ps)
        wprods.append(wp)

    indeg = [0] * n
    succ: list = [[] for _ in range(n)]
    for k in range(n):
        for p in preds[k]:
            succ[p].append(k)
            indeg[k] += 1
    from collections import deque

    topo, q = [], deque(k for k in range(n) if indeg[k] == 0)
    while q:
        k = q.popleft()
        topo.append(k)
        for s2 in succ[k]:
            indeg[s2] -= 1
            if indeg[s2] == 0:
                q.append(s2)
    if len(topo) != n:
        return  # unexpected cycle: leave program untouched

    F = [0] * n  # bitset: instructions known COMPLETED when inst k issues
    C = [0] * n  # completed-set implied by inst k's completion (incl. itself)
    stream_pred: dict = {}
    seen_eng: dict = {}
    for k, i in enumerate(insts):
        eng = i.engine
        stream_pred[k] = seen_eng.get(eng)
        seen_eng[eng] = k

    for k in topo:
        i = insts[k]
        waits = all_waits[k]
        sp = stream_pred[k]
        base = F[sp] if sp is not None else 0
        wsets = []
        for ps in wprods[k]:
            if ps is None:
                wsets.append(None)
            else:
                s = 0
                for p in ps:
                    s |= C[p]
                wsets.append(s)
        keep = list(range(len(waits)))
        changed = True
        while changed:
            changed = False
            for j in list(keep):
                w = waits[j]
                if wprods[k][j] is None:
                    continue
                known = base
                for j2 in keep:
                    if j2 != j and wsets[j2] is not None:
                        known |= wsets[j2]
                credit = 0
                for ki, v in upd.get(w.ant_name, []):
                    if v is not None and (known >> ki) & 1:
                        credit += v
                if credit >= w.wait_value:
                    keep.remove(j)
                    changed = True
                    break
        fac = base
        for j in keep:
            if wsets[j] is not None:
                fac |= wsets[j]
        F[k] = fac
        C[k] = fac | (1 << k)
        if len(keep) != len(waits):
            si = i.sync_info
            i.sync_info = mybir.SyncInfo(
                on_wait=[waits[j] for j in keep], on_update=list(si.on_update or [])
            )


_CACHE: dict = {}
last_result = None  # BassKernelResults of the most recent run (for profiling)


def kernel(eps: np.ndarray, N=None, **_unused) -> np.ndarray:
    global last_result
    eps = np.ascontiguousarray(np.asarray(eps, dtype=np.float32))
    n = eps.shape[0]
    assert n % N_CORES == 0 and eps.shape[1] == TOTAL
    shard = n // N_CORES

    nc = _CACHE.get(("nc", shard))
    if nc is None:
        nc = _CACHE[("nc", shard)] = build_nc(rows=shard)

    in_maps = [{"eps": eps[i * shard : (i + 1) * shard]} for i in range(N_CORES)]
    res = run_bass_kernel_spmd(nc, in_maps, list(range(N_CORES)))
    last_result = res
    return np.concatenate([r["out"] for r in res.results], axis=0)
